# revision 61
# baseline (speedup 1.0000x reference)
"""Causal GQA attention block (B=2,S=2048,D=1024,H=16,KH=4,DK=64) on 8 TRN2 cores.

Sharding: core c -> (batch b=c//4, kv-group g=c%4). Each core computes its
batch's 4 query heads (one kv head); Wq/Wk/Wv column-parallel, Wo
row-parallel; per-core partial outputs (out^T, fp16) are summed on host.

All matmul inputs are bf16 (inputs converted host-side); PSUM stays fp32.
Device algorithm per core, software-pipelined across the 4 sequence chunks
(QT=512): proj+RoPE chunk c feeds causal attention q-tile c; proj of chunk
c+1 and the out-projection of q-tile c-1 are interleaved into q-tile c's
score/PV sweeps so the in-order PE queue never waits on the ACT engine's
exp stream. Attention runs per parity (partition-half: heads {0,2} then
{1,3}) with 2 head-slots along the free dim, scores S^T [128k, n] blocks,
exp on ACT -> bf16 P, triangular masks via gpsimd affine_select, P^T @ V_aug
(ones column gives softmax denominators).
"""

import sys

sys.path.insert(0, "/opt/trn_rl_repo")

import numpy as np
import ml_dtypes

import concourse.bass as bass
import concourse.bacc as bacc
import concourse.mybir as mybir
from concourse import library_config
from concourse.bass_utils import run_bass_kernel_spmd
from concourse.masks import make_identity
from concourse.tile import TileContext

F32 = mybir.dt.float32
F16 = mybir.dt.float16
BF16 = mybir.dt.bfloat16
BF = ml_dtypes.bfloat16
EXP = mybir.ActivationFunctionType.Exp
MULT = mybir.AluOpType.mult
GE = mybir.AluOpType.is_ge

B, S, D = 2, 2048, 1024
H, KH, DK = 16, 4, 64
REP = H // KH  # query heads per kv head / per core
GDIM = REP * DK  # 256 query-proj columns per core
HALF = DK // 2  # 32
SCALE = 1.0 / np.sqrt(DK)

QT = 512  # q-tile
KB = 128  # k-block
NQT = S // QT  # 4
ND = D // 128  # 8 contraction tiles for projections


def build_nc() -> bass.Bass:
    nc = bacc.Bacc("TRN2", target_bir_lowering=False, debug=False)

    xt_d = nc.declare_dram_parameter("xt", [D, S], BF16, isOutput=False)
    wq0_d = nc.declare_dram_parameter("wq0", [128, ND, 128], BF16,
                                      isOutput=False)
    wq1_d = nc.declare_dram_parameter("wq1", [128, ND, 128], BF16,
                                      isOutput=False)
    wvk_d = nc.declare_dram_parameter("wvk", [128, ND, 128], BF16,
                                      isOutput=False)
    wo_d = nc.declare_dram_parameter("wo", [128, 2, D], BF16, isOutput=False)
    cos_d = nc.declare_dram_parameter("cosq", [128, S], BF16, isOutput=False)
    sin_d = nc.declare_dram_parameter("sinq", [128, S], BF16, isOutput=False)
    out_d = nc.declare_dram_parameter("outT", [D, S], F16, isOutput=True)

    xt_r = xt_d.rearrange("(t p) s -> p t s", p=128)
    out_r = out_d.rearrange("(t p) s -> p t s", p=128)

    with TileContext(nc) as tc:
        with tc.tile_pool(name="persist", bufs=1) as pp, \
             tc.tile_pool(name="qraw", bufs=6) as qraw_pool, \
             tc.tile_pool(name="pt", bufs=8) as pt_pool, \
             tc.tile_pool(name="small", bufs=4) as ssb, \
             tc.tile_pool(name="ob", bufs=12) as ob_pool, \
             tc.tile_pool(name="sc_ps", bufs=2, space="PSUM") as sc_ps, \
             tc.tile_pool(name="st_ps", bufs=2, space="PSUM") as st_ps, \
             tc.tile_pool(name="ot_ps", bufs=2, space="PSUM") as ot_ps:

            # ---------------- persistent SBUF state ----------------
            w_sb = {n: pp.tile([128, ND, 128], BF16, tag=n, name=n)
                    for n in ("wq0", "wq1", "wvk")}
            w_dram = {"wq0": wq0_d, "wq1": wq1_d, "wvk": wvk_d}

            def load_w(name):
                nc.sync.dma_start(w_sb[name][:], w_dram[name][:])

            xt_sb = pp.tile([128, ND, S], BF16, tag="xt")
            cos_sb = pp.tile([128, S], BF16, tag="cos")
            sin_sb = pp.tile([128, S], BF16, tag="sin")
            wo_sb = pp.tile([128, 2, D], BF16, tag="wo")
            ident = pp.tile([64, 64], BF16, tag="ident")
            zbias = pp.tile([128, 1], F32, tag="zbias")
            # qh[p, s, :]: partitions 0:64 head s (even parity: heads 0/2),
            # partitions 64:128 head s+? (odd parity: heads 1/3); slot s in
            # {0,1} -> q-stack s (heads 2s, 2s+1)
            qh_sb = pp.tile([128, 2, S], BF16, tag="qh")
            kt2_sb = pp.tile([128, S], BF16, tag="kt2")  # roped K^T, dup rows
            vt_sb = pp.tile([64, S], BF16, tag="vt")     # V^T
            vaug_sb = pp.tile([128, S // KB, 66], BF16, tag="vaug")
            at_sb = pp.tile([128, 2, S], BF16, tag="at")

            # ---------------- preamble ----------------
            nc.sync.dma_start(w_sb["wq0"][:, 0:2, :], wq0_d[:, 0:2, :])
            nc.sync.dma_start(xt_sb[:, 0:2, 0:QT], xt_r[:, 0:2, 0:QT])
            nc.sync.dma_start(w_sb["wq0"][:, 2:ND, :], wq0_d[:, 2:ND, :])
            nc.sync.dma_start(xt_sb[:, 2:4, 0:QT], xt_r[:, 2:4, 0:QT])
            load_w("wq1")
            nc.sync.dma_start(xt_sb[:, 4:6, 0:QT], xt_r[:, 4:6, 0:QT])
            nc.sync.dma_start(xt_sb[:, 6:8, 0:QT], xt_r[:, 6:8, 0:QT])
            nc.sync.dma_start(cos_sb[:, 0:QT], cos_d[:, 0:QT])
            nc.sync.dma_start(sin_sb[:, 0:QT], sin_d[:, 0:QT])
            load_w("wvk")
            nc.sync.dma_start(cos_sb[:, QT:S], cos_d[:, QT:S])
            nc.sync.dma_start(sin_sb[:, QT:S], sin_d[:, QT:S])
            nc.sync.dma_start(wo_sb[:], wo_d[:])
            make_identity(nc, ident[:])
            nc.vector.memset(zbias[:], 0.0)
            nc.vector.memset(vaug_sb[:, :, 64], 1.0)
            nc.gpsimd.load_library(library_config.attn)

            # ---------------- pipeline building blocks ----------------
            raws, pss = {}, {}

            def proj_stack_mm(stack, c, half):
                """Phase A (half 0/1): 4 proj matmuls; half 1 also
                evacuates PSUM."""
                cs = slice(c * QT, (c + 1) * QT)
                if half == 0:
                    pss[stack] = sc_ps.tile([128, QT], F32, tag="sc",
                                            name="proj")
                ps = pss[stack]
                for t in range(4 * half, 4 * half + 4):
                    nc.tensor.matmul(
                        ps[:], w_sb[stack][:, t, :], xt_sb[:, t, cs],
                        start=(t == 0), stop=(t == ND - 1))
                if half == 1:
                    del pss[stack]
                    raw = qraw_pool.tile([128, QT], BF16, tag="qraw",
                                         name="qraw")
                    if c == 0:  # ACT is idle during startup
                        nc.scalar.copy(raw[:], ps[:])
                        if stack == "wvk":
                            nc.scalar.copy(vt_sb[:, cs], ps[0:64, :])
                    else:
                        nc.vector.tensor_copy(raw[:], ps[:])
                        if stack == "wvk":
                            nc.vector.tensor_copy(vt_sb[:, cs], ps[0:64, :])
                    raws[stack] = raw

            def proj_stack_rope(stack, c):
                """Phase B: rotate-half on PE (into the op psum ring) +
                elementwise rope on DVE."""
                cs = slice(c * QT, (c + 1) * QT)
                raw = raws.pop(stack)
                if stack == "wvk":
                    lo, hi = 64, 128
                    dst = kt2_sb[64:128, cs]
                else:
                    lo, hi = 0, 128
                    s = 0 if stack == "wq0" else 1
                    dst = qh_sb[:, s, cs]
                # rotate-half via DVE lane shuffle (dims are stored in a
                # host-side order where the rope partner is 16 lanes away
                # inside each 32-partition group; sign lives in sinq)
                swp = qraw_pool.tile([128, QT], BF16, tag="swp", name="swp")
                nc.vector.stream_shuffle(
                    swp[lo:hi, :], raw[lo:hi, :],
                    [(i + 16) % 32 for i in range(32)])
                t2 = qraw_pool.tile([128, QT], BF16, tag="t2", name="t2")
                nc.vector.tensor_tensor(t2[lo:hi, :], swp[lo:hi, :],
                                        sin_sb[lo:hi, cs], MULT)
                nc.gpsimd.tensor_tensor(raw[lo:hi, :], raw[lo:hi, :],
                                        cos_sb[lo:hi, cs], MULT)
                nc.gpsimd.tensor_add(dst, raw[lo:hi, :], t2[lo:hi, :])
                if stack == "wvk":
                    # duplicate roped K to partitions 0:64 for even parity
                    nc.sync.dma_start(kt2_sb[0:64, cs], kt2_sb[64:128, cs])

            def vaug_block(kt):
                """V_aug tile for k-block kt via PE transpose (bf16)."""
                tp = sc_ps.tile([128, QT], F32, tag="sc", name="tp")
                tpb = tp[:].bitcast(BF16)
                nc.tensor.transpose(
                    tpb[:, 0:64], vt_sb[:, kt * KB:(kt + 1) * KB], ident[:])
                nc.vector.tensor_copy(vaug_sb[:, kt, 0:64], tpb[:, 0:64])

            def proj_chunk_tasks(c):
                """Thunks for chunk c: A/B phases interleaved so phase B
                never makes the in-order PE queue wait on the evacuation."""
                yield lambda: proj_stack_mm("wq0", c, 0)
                yield lambda: proj_stack_mm("wq0", c, 1)
                yield lambda: proj_stack_mm("wq1", c, 0)
                yield lambda: proj_stack_mm("wq1", c, 1)
                yield lambda: proj_stack_rope("wq0", c)
                yield lambda: proj_stack_mm("wvk", c, 0)
                yield lambda: proj_stack_mm("wvk", c, 1)
                yield lambda: proj_stack_rope("wq1", c)
                yield lambda: proj_stack_rope("wvk", c)
                for kt in range(4 * c, 4 * c + 4):
                    yield lambda k=kt: vaug_block(k)

            def outproj_tasks(qt, tail=False):
                """Generator of thunks: out-projection for q-tile qt, one
                dc-tile per thunk. In the tail, evacuate on the otherwise
                idle ACT engine."""
                if not tail:
                    for dc in range(ND):
                        def task(dc=dc, qt=qt):
                            qs = slice(qt * QT, (qt + 1) * QT)
                            ob = ob_pool.tile([128, QT], F16, tag="ob",
                                              name="ob")
                            op = sc_ps.tile([128, QT], F32, tag="sc",
                                            name="op")
                            for s in range(2):
                                nc.tensor.matmul(
                                    op[:],
                                    wo_sb[:, s, dc * 128:(dc + 1) * 128],
                                    at_sb[:, s, qs],
                                    start=(s == 0), stop=(s == 1))
                            nc.vector.tensor_copy(ob[:], op[:])
                            nc.sync.dma_start(out_r[:, dc, qs], ob[:])
                        yield task
                    return
                # tail: half-columns h-major (all h0 thunks, then h1) so
                # the h1 norm never blocks h0 work in the in-order PE
                # queue; op buffers alternate sc/st pools per dc; evacs
                # alternate ACT/DVE into a persistent staging buffer,
                # DMA'd out in 4-dc batches.
                HQ = QT // 2
                obt = pp.tile([128, ND, QT], F16, tag="obt")
                for h in range(2):
                    for dc in range(ND):
                        def task(dc=dc, h=h, qt=qt):
                            qs = slice(qt * QT + h * HQ,
                                       qt * QT + (h + 1) * HQ)
                            if dc % 2 == 0:
                                op = sc_ps.tile([128, QT], F32, tag="sc",
                                                name="op")[:, 0:HQ]
                            else:
                                op = st_ps.tile([128, 2, QT], F32, tag="st",
                                                name="op")[:, 0, 0:HQ]
                            for s in range(2):
                                nc.tensor.matmul(
                                    op,
                                    wo_sb[:, s, dc * 128:(dc + 1) * 128],
                                    at_sb[:, s, qs],
                                    start=(s == 0), stop=(s == 1))
                            dst = obt[:, dc, h * HQ:(h + 1) * HQ]
                            if dc % 2 == 0:
                                nc.scalar.copy(dst, op)
                            else:
                                nc.vector.tensor_copy(dst, op)
                            nb = 4 if h == 1 else 2
                            if dc % nb == nb - 1:  # batched DMA
                                nc.sync.dma_start(
                                    out_r[:, dc - nb + 1:dc + 1, qs],
                                    obt[:, dc - nb + 1:dc + 1,
                                        h * HQ:(h + 1) * HQ])
                        yield task

            # ---------------- attention sweep ----------------
            def attention_qt(qt, fillers, parity_order=(0, 1)):
                """Causal attention for q-tile qt, both parities; pulls one
                filler thunk (proj/outproj work) every block."""
                nblk = 4 * qt + 4
                q0 = qt * QT

                npulls = 2 * (nblk + 2)
                state = {"done": 0, "credit": 0.0}
                rate = None

                def pull():
                    if not fillers:
                        return
                    if rate is None:
                        fillers.pop(0)()
                        return
                    state["credit"] += rate
                    while fillers and state["credit"] >= 1.0:
                        state["credit"] -= 1.0
                        fillers.pop(0)()

                rate = len(fillers) / float(npulls)
                for par in parity_order:  # 0: heads {0,2} rows 0:64
                    rows = slice(64 * par, 64 * par + 64)
                    ots = [ot_ps.tile([65, QT], F32, tag="ot", name="ot")
                           for _ in range(2)]
                    sts, pts, offs = {}, {}, {}

                    def scores(kb):
                        moff = max(0, (kb - 4 * qt) * KB)
                        n = QT - moff
                        st = st_ps.tile([128, 2, QT], F32, tag="st",
                                        name="st")
                        for s in range(2):
                            nc.tensor.matmul(
                                st[:, s, moff:QT],
                                kt2_sb[rows, kb * KB:(kb + 1) * KB],
                                qh_sb[rows, s, q0 + moff:q0 + QT],
                                start=True, stop=True)
                        pt = pt_pool.tile([128, 2, QT], BF16, tag="pt",
                                          name="pt")
                        nc.scalar.activation(
                            pt[:, :, moff:QT], st[:, :, moff:QT], EXP,
                            scale=float(SCALE), bias=zbias[:])
                        if kb >= 4 * qt:  # diagonal block: causal mask
                            nc.gpsimd.affine_select(
                                out=pt[:, :, moff:moff + KB],
                                in_=pt[:, :, moff:moff + KB],
                                compare_op=GE, fill=0.0, base=0,
                                pattern=[[0, 2], [1, KB]],
                                channel_multiplier=-1)
                        sts[kb], pts[kb], offs[kb] = st, pt, moff

                    def pv(kb):
                        pt, moff = pts.pop(kb), offs[kb]
                        del sts[kb]
                        for s in range(2):
                            nc.tensor.matmul(
                                ots[s][:, moff:QT],
                                vaug_sb[:, kb, 0:65],
                                pt[:, s, moff:QT],
                                start=(kb == 0), stop=(kb == nblk - 1))

                    scores(0)
                    if nblk > 1:
                        scores(1)
                    state["credit"] += 1.0
                    pull()
                    pull()
                    for kb in range(nblk):
                        if kb + 2 < nblk:
                            scores(kb + 2)
                        else:
                            pull()
                        pv(kb)
                        pull()
                    state["credit"] += 1.5
                    pull()
                    pull()

                    # normalize per slot: at = ot[0:64] * (1 / ot[64]);
                    # in the tail parity, split by (half, slot) h-major so
                    # the tail out-projection's first half starts early
                    tailpar = (qt == NQT - 1 and par == parity_order[1])
                    pieces = [(h, s) for h in range(2) for s in range(2)] \
                        if tailpar else [(None, s) for s in range(2)]
                    atmp = None
                    if par == 1:
                        atmp = ssb.tile([64, 2, QT], BF16, tag="atmp",
                                        name="atmp")
                    for h, s in pieces:
                        hs = slice(0, QT) if h is None else \
                            slice(h * (QT // 2), (h + 1) * (QT // 2))
                        nw = hs.stop - hs.start
                        lrec = ssb.tile([1, QT], F32, tag="lrec",
                                        name="lrec")[:, 0:nw]
                        nc.vector.reciprocal(lrec, ots[s][64:65, hs])
                        lrecb = ssb.tile([64, QT], F32, tag="lrecb",
                                         name="lrecb")[:, 0:nw]
                        nc.gpsimd.partition_broadcast(lrecb, lrec)
                        if par == 0:
                            nc.vector.tensor_tensor(
                                at_sb[0:64, s, q0 + hs.start:q0 + hs.stop],
                                ots[s][0:64, hs], lrecb, MULT)
                        else:
                            nc.vector.tensor_tensor(atmp[:, s, hs],
                                                    ots[s][0:64, hs],
                                                    lrecb, MULT)
                    if par == 1:
                        nc.sync.dma_start(at_sb[64:128, :, q0:q0 + QT],
                                          atmp[:])

            # ---------------- emit the pipeline ----------------
            for task in proj_chunk_tasks(0):
                task()
            for c in range(NQT):
                if c + 1 < NQT:
                    cs = slice((c + 1) * QT, (c + 2) * QT)
                    nc.sync.dma_start(xt_sb[:, 0:4, cs], xt_r[:, 0:4, cs])
                    nc.sync.dma_start(xt_sb[:, 4:ND, cs], xt_r[:, 4:ND, cs])
                fillers = []
                if c + 1 < NQT:
                    fillers.extend(proj_chunk_tasks(c + 1))
                if c == 1:
                    fillers.extend(outproj_tasks(0))
                elif c == 3:
                    op1 = list(outproj_tasks(1))
                    op2 = list(outproj_tasks(2))
                    # interleave the two qts' outproj thunks
                    for a, b in zip(op1, op2):
                        fillers.extend((a, b))
                attention_qt(c, fillers,
                             parity_order=(1, 0) if c == NQT - 1 else (0, 1))
                for f in fillers:  # anything not pulled during the sweep
                    f()
            for task in outproj_tasks(NQT - 1, tail=True):
                task()
    nc.compile()
    return nc


_NC_CACHE = None


def _get_nc():
    global _NC_CACHE
    if _NC_CACHE is None:
        _NC_CACHE = build_nc()
    return _NC_CACHE


# Q/K head dims are stored permuted so the rotate-half partner of the dim
# at lane p sits at lane (p+16)%32 within p's 32-lane group (DVE
# stream_shuffle reach). Scores are invariant to the shared permutation.
PERM64 = np.concatenate([np.arange(0, 16), np.arange(32, 48),
                         np.arange(16, 32), np.arange(48, 64)])


def _rope_tables():
    theta = 10000.0 ** (-(np.arange(HALF, dtype=np.float64) / HALF))
    pos = np.arange(S, dtype=np.float64)
    freqs = pos[:, None] * theta[None, :]  # [S, 32]
    cos1 = np.cos(freqs).T  # [32, S]
    sin1 = np.sin(freqs).T
    sign = np.where(PERM64 < HALF, -1.0, 1.0)[:, None]
    cos64 = cos1[PERM64 % HALF]
    sin64 = sign * sin1[PERM64 % HALF]
    cosq = np.tile(cos64, (2, 1)).astype(BF)  # [128, S]
    sinq = np.tile(sin64, (2, 1)).astype(BF)
    return np.ascontiguousarray(cosq), np.ascontiguousarray(sinq)


def _permute_qk_cols(w):
    """Permute each head's 64 columns of a [D, n*64] Q/K weight slice."""
    n = w.shape[1] // 64
    idx = np.concatenate([h * 64 + PERM64 for h in range(n)])
    return w[:, idx]


def _tp(w):
    """[t*128, m] -> [128, t, m] (pre-rearranged for big DMA descriptors)."""
    t = w.shape[0] // 128
    return np.ascontiguousarray(
        w.reshape(t, 128, w.shape[1]).transpose(1, 0, 2))


def make_in_maps(x, Wq, Wk, Wv, Wo):
    cosq, sinq = _rope_tables()
    xts = [np.ascontiguousarray(x[b].T.astype(BF)) for b in range(B)]
    wslices = {}
    in_maps = []
    for c in range(8):
        b, g = divmod(c, 4)
        if g not in wslices:
            wslices[g] = {
                "wq0": _tp(_permute_qk_cols(
                    Wq[:, g * GDIM:g * GDIM + 128]).astype(BF)),
                "wq1": _tp(_permute_qk_cols(
                    Wq[:, g * GDIM + 128:(g + 1) * GDIM]).astype(BF)),
                "wvk": _tp(np.concatenate(
                    [Wv[:, g * DK:(g + 1) * DK],
                     _permute_qk_cols(Wk[:, g * DK:(g + 1) * DK])],
                    axis=1).astype(BF)),
                "wo": _tp(Wo[g * GDIM:(g + 1) * GDIM, :].astype(BF)),
            }
        in_maps.append({
            "xt": xts[b], **wslices[g],
            "cosq": cosq, "sinq": sinq,
        })
    return in_maps


def kernel(x, mask, Wq, bq, Wk, bk, Wv, bv, Wo, bo):
    x = np.asarray(x, dtype=np.float32)
    mask = np.asarray(mask)
    Wq, Wk, Wv, Wo = (np.asarray(w, dtype=np.float32) for w in (Wq, Wk, Wv, Wo))
    bq, bk, bv, bo = (np.asarray(b, dtype=np.float32) for b in (bq, bk, bv, bo))

    assert np.array_equal(
        np.asarray(mask[0, 0]), np.tril(np.ones((S, S), mask.dtype))
    ), "kernel specialized for the causal mask"
    assert not bq.any() and not bk.any(), (
        "nonzero bq/bk not supported (cannot be folded outside RoPE)"
    )

    in_maps = make_in_maps(x, Wq, Wk, Wv, Wo)
    res = run_bass_kernel_spmd(_get_nc(), in_maps, list(range(8)))
    out = np.zeros((B, S, D), dtype=np.float32)
    for c in range(8):
        out[c // 4] += res.results[c]["outT"].astype(np.float32).T
    # host-side fold of the (structurally zero) v/out biases:
    # rows of softmax(P) sum to 1, so P @ (V + 1 bv^T) @ Wo + bo
    #   = P@V@Wo + sum_g bv_g_expanded @ Wo_g + bo
    corr = bo.astype(np.float64).copy()
    if bv.any():
        for g in range(KH):
            bv_full = np.tile(bv[g * DK:(g + 1) * DK], REP)
            corr = corr + bv_full.astype(np.float64) @ Wo[g * GDIM:(g + 1) * GDIM]
    if corr.any():
        out = out + corr[None, None, :].astype(np.float32)
    return out


# revision 63
# speedup vs baseline: 1.0001x; 1.0001x over previous
"""Causal GQA attention block (B=2,S=2048,D=1024,H=16,KH=4,DK=64) on 8 TRN2 cores.

Sharding: core c -> (batch b=c//4, kv-group g=c%4). Each core computes its
batch's 4 query heads (one kv head); Wq/Wk/Wv column-parallel, Wo
row-parallel; per-core partial outputs (out^T, fp16) are summed on host.

All matmul inputs are bf16 (inputs converted host-side); PSUM stays fp32.
Device algorithm per core, software-pipelined across the 4 sequence chunks
(QT=512): proj+RoPE chunk c feeds causal attention q-tile c; proj of chunk
c+1 and the out-projection of q-tile c-1 are interleaved into q-tile c's
score/PV sweeps so the in-order PE queue never waits on the ACT engine's
exp stream. Attention runs per parity (partition-half: heads {0,2} then
{1,3}) with 2 head-slots along the free dim, scores S^T [128k, n] blocks,
exp on ACT -> bf16 P, triangular masks via gpsimd affine_select, P^T @ V_aug
(ones column gives softmax denominators).
"""

import sys

sys.path.insert(0, "/opt/trn_rl_repo")

import numpy as np
import ml_dtypes

import concourse.bass as bass
import concourse.bacc as bacc
import concourse.mybir as mybir
from concourse import library_config
from concourse.bass_utils import run_bass_kernel_spmd
from concourse.masks import make_identity
from concourse.tile import TileContext

F32 = mybir.dt.float32
F16 = mybir.dt.float16
BF16 = mybir.dt.bfloat16
BF = ml_dtypes.bfloat16
EXP = mybir.ActivationFunctionType.Exp
MULT = mybir.AluOpType.mult
GE = mybir.AluOpType.is_ge

B, S, D = 2, 2048, 1024
H, KH, DK = 16, 4, 64
REP = H // KH  # query heads per kv head / per core
GDIM = REP * DK  # 256 query-proj columns per core
HALF = DK // 2  # 32
SCALE = 1.0 / np.sqrt(DK)

QT = 512  # q-tile
KB = 128  # k-block
NQT = S // QT  # 4
ND = D // 128  # 8 contraction tiles for projections


def build_nc() -> bass.Bass:
    nc = bacc.Bacc("TRN2", target_bir_lowering=False, debug=False)

    xt_d = nc.declare_dram_parameter("xt", [D, S], BF16, isOutput=False)
    wq0_d = nc.declare_dram_parameter("wq0", [128, ND, 128], BF16,
                                      isOutput=False)
    wq1_d = nc.declare_dram_parameter("wq1", [128, ND, 128], BF16,
                                      isOutput=False)
    wvk_d = nc.declare_dram_parameter("wvk", [128, ND, 128], BF16,
                                      isOutput=False)
    wo_d = nc.declare_dram_parameter("wo", [128, 2, D], BF16, isOutput=False)
    cos_d = nc.declare_dram_parameter("cosq", [128, S], BF16, isOutput=False)
    sin_d = nc.declare_dram_parameter("sinq", [128, S], BF16, isOutput=False)
    out_d = nc.declare_dram_parameter("outT", [D, S], F16, isOutput=True)

    xt_r = xt_d.rearrange("(t p) s -> p t s", p=128)
    out_r = out_d.rearrange("(t p) s -> p t s", p=128)

    with TileContext(nc) as tc:
        with tc.tile_pool(name="persist", bufs=1) as pp, \
             tc.tile_pool(name="qraw", bufs=6) as qraw_pool, \
             tc.tile_pool(name="pt", bufs=8) as pt_pool, \
             tc.tile_pool(name="small", bufs=4) as ssb, \
             tc.tile_pool(name="ob", bufs=12) as ob_pool, \
             tc.tile_pool(name="sc_ps", bufs=2, space="PSUM") as sc_ps, \
             tc.tile_pool(name="st_ps", bufs=2, space="PSUM") as st_ps, \
             tc.tile_pool(name="ot_ps", bufs=2, space="PSUM") as ot_ps:

            # ---------------- persistent SBUF state ----------------
            w_sb = {n: pp.tile([128, ND, 128], BF16, tag=n, name=n)
                    for n in ("wq0", "wq1", "wvk")}
            w_dram = {"wq0": wq0_d, "wq1": wq1_d, "wvk": wvk_d}

            def load_w(name):
                nc.sync.dma_start(w_sb[name][:], w_dram[name][:])

            xt_sb = pp.tile([128, ND, S], BF16, tag="xt")
            cos_sb = pp.tile([128, S], BF16, tag="cos")
            sin_sb = pp.tile([128, S], BF16, tag="sin")
            wo_sb = pp.tile([128, 2, D], BF16, tag="wo")
            ident = pp.tile([64, 64], BF16, tag="ident")
            zbias = pp.tile([128, 1], F32, tag="zbias")
            # qh[p, s, :]: partitions 0:64 head s (even parity: heads 0/2),
            # partitions 64:128 head s+? (odd parity: heads 1/3); slot s in
            # {0,1} -> q-stack s (heads 2s, 2s+1)
            qh_sb = pp.tile([128, 2, S], BF16, tag="qh")
            kt2_sb = pp.tile([128, S], BF16, tag="kt2")  # roped K^T, dup rows
            vt_sb = pp.tile([64, S], BF16, tag="vt")     # V^T
            vaug_sb = pp.tile([128, S // KB, 66], BF16, tag="vaug")
            at_sb = pp.tile([128, 2, S], BF16, tag="at")

            # ---------------- preamble ----------------
            nc.sync.dma_start(w_sb["wq0"][:, 0:2, :], wq0_d[:, 0:2, :])
            nc.sync.dma_start(xt_sb[:, 0:2, 0:QT], xt_r[:, 0:2, 0:QT])
            nc.sync.dma_start(w_sb["wq0"][:, 2:ND, :], wq0_d[:, 2:ND, :])
            nc.sync.dma_start(xt_sb[:, 2:4, 0:QT], xt_r[:, 2:4, 0:QT])
            load_w("wq1")
            nc.sync.dma_start(xt_sb[:, 4:6, 0:QT], xt_r[:, 4:6, 0:QT])
            nc.sync.dma_start(xt_sb[:, 6:8, 0:QT], xt_r[:, 6:8, 0:QT])
            nc.sync.dma_start(cos_sb[:, 0:QT], cos_d[:, 0:QT])
            nc.sync.dma_start(sin_sb[:, 0:QT], sin_d[:, 0:QT])
            load_w("wvk")
            nc.sync.dma_start(cos_sb[:, QT:S], cos_d[:, QT:S])
            nc.sync.dma_start(sin_sb[:, QT:S], sin_d[:, QT:S])
            nc.sync.dma_start(wo_sb[:], wo_d[:])
            make_identity(nc, ident[:])
            nc.vector.memset(zbias[:], 0.0)
            nc.vector.memset(vaug_sb[:, :, 64], 1.0)
            nc.gpsimd.load_library(library_config.attn)

            # ---------------- pipeline building blocks ----------------
            raws, pss = {}, {}

            def proj_stack_mm(stack, c, half):
                """Phase A (half 0/1): 4 proj matmuls; half 1 also
                evacuates PSUM."""
                cs = slice(c * QT, (c + 1) * QT)
                if half == 0:
                    pss[stack] = sc_ps.tile([128, QT], F32, tag="sc",
                                            name="proj")
                ps = pss[stack]
                for t in range(4 * half, 4 * half + 4):
                    nc.tensor.matmul(
                        ps[:], w_sb[stack][:, t, :], xt_sb[:, t, cs],
                        start=(t == 0), stop=(t == ND - 1))
                if half == 1:
                    del pss[stack]
                    raw = qraw_pool.tile([128, QT], BF16, tag="qraw",
                                         name="qraw")
                    if c == 0:  # ACT is idle during startup
                        nc.scalar.copy(raw[:], ps[:])
                        if stack == "wvk":
                            nc.scalar.copy(vt_sb[:, cs], ps[0:64, :])
                    else:
                        nc.vector.tensor_copy(raw[:], ps[:])
                        if stack == "wvk":
                            nc.vector.tensor_copy(vt_sb[:, cs], ps[0:64, :])
                    raws[stack] = raw

            def proj_stack_rope(stack, c):
                """Phase B: rotate-half on PE (into the op psum ring) +
                elementwise rope on DVE."""
                cs = slice(c * QT, (c + 1) * QT)
                raw = raws.pop(stack)
                if stack == "wvk":
                    lo, hi = 64, 128
                    dst = kt2_sb[64:128, cs]
                else:
                    lo, hi = 0, 128
                    s = 0 if stack == "wq0" else 1
                    dst = qh_sb[:, s, cs]
                # rotate-half via DVE lane shuffle (dims are stored in a
                # host-side order where the rope partner is 16 lanes away
                # inside each 32-partition group; sign lives in sinq)
                swp = qraw_pool.tile([128, QT], BF16, tag="swp", name="swp")
                nc.vector.stream_shuffle(
                    swp[lo:hi, :], raw[lo:hi, :],
                    [(i + 16) % 32 for i in range(32)])
                t2 = qraw_pool.tile([128, QT], BF16, tag="t2", name="t2")
                nc.vector.tensor_tensor(t2[lo:hi, :], swp[lo:hi, :],
                                        sin_sb[lo:hi, cs], MULT)
                nc.gpsimd.tensor_tensor(raw[lo:hi, :], raw[lo:hi, :],
                                        cos_sb[lo:hi, cs], MULT)
                nc.gpsimd.tensor_add(dst, raw[lo:hi, :], t2[lo:hi, :])
                if stack == "wvk":
                    # duplicate roped K to partitions 0:64 for even parity
                    nc.sync.dma_start(kt2_sb[0:64, cs], kt2_sb[64:128, cs])

            def vaug_block(kt):
                """V_aug tile for k-block kt via PE transpose (bf16)."""
                tp = sc_ps.tile([128, QT], F32, tag="sc", name="tp")
                tpb = tp[:].bitcast(BF16)
                nc.tensor.transpose(
                    tpb[:, 0:64], vt_sb[:, kt * KB:(kt + 1) * KB], ident[:])
                nc.vector.tensor_copy(vaug_sb[:, kt, 0:64], tpb[:, 0:64])

            def proj_chunk_tasks(c):
                """Thunks for chunk c: A/B phases interleaved so phase B
                never makes the in-order PE queue wait on the evacuation."""
                yield lambda: proj_stack_mm("wq0", c, 0)
                yield lambda: proj_stack_mm("wq0", c, 1)
                yield lambda: proj_stack_mm("wq1", c, 0)
                yield lambda: proj_stack_mm("wq1", c, 1)
                yield lambda: proj_stack_rope("wq0", c)
                yield lambda: proj_stack_mm("wvk", c, 0)
                yield lambda: proj_stack_mm("wvk", c, 1)
                yield lambda: proj_stack_rope("wq1", c)
                yield lambda: proj_stack_rope("wvk", c)
                for kt in range(4 * c, 4 * c + 4):
                    yield lambda k=kt: vaug_block(k)

            def outproj_tasks(qt, tail=False):
                """Generator of thunks: out-projection for q-tile qt, one
                dc-tile per thunk. In the tail, evacuate on the otherwise
                idle ACT engine."""
                if not tail:
                    for dc in range(ND):
                        def task(dc=dc, qt=qt):
                            qs = slice(qt * QT, (qt + 1) * QT)
                            ob = ob_pool.tile([128, QT], F16, tag="ob",
                                              name="ob")
                            op = sc_ps.tile([128, QT], F32, tag="sc",
                                            name="op")
                            for s in range(2):
                                nc.tensor.matmul(
                                    op[:],
                                    wo_sb[:, s, dc * 128:(dc + 1) * 128],
                                    at_sb[:, s, qs],
                                    start=(s == 0), stop=(s == 1))
                            nc.vector.tensor_copy(ob[:], op[:])
                            nc.sync.dma_start(out_r[:, dc, qs], ob[:])
                        yield task
                    return
                # tail: half-columns h-major (all h0 thunks, then h1) so
                # the h1 norm never blocks h0 work in the in-order PE
                # queue; op buffers alternate sc/st pools per dc; evacs
                # alternate ACT/DVE into a persistent staging buffer,
                # DMA'd out in 4-dc batches.
                HQ = QT // 2
                obt = pp.tile([128, ND, QT], F16, tag="obt")
                for h in range(2):
                    for dc in range(ND):
                        def task(dc=dc, h=h, qt=qt):
                            qs = slice(qt * QT + h * HQ,
                                       qt * QT + (h + 1) * HQ)
                            if dc % 2 == 0:
                                op = sc_ps.tile([128, QT], F32, tag="sc",
                                                name="op")[:, 0:HQ]
                            else:
                                op = st_ps.tile([128, 2, QT], F32, tag="st",
                                                name="op")[:, 0, 0:HQ]
                            for s in range(2):
                                nc.tensor.matmul(
                                    op,
                                    wo_sb[:, s, dc * 128:(dc + 1) * 128],
                                    at_sb[:, s, qs],
                                    start=(s == 0), stop=(s == 1))
                            dst = obt[:, dc, h * HQ:(h + 1) * HQ]
                            if dc % 2 == 0:
                                nc.scalar.copy(dst, op)
                            else:
                                nc.vector.tensor_copy(dst, op)
                            nb = 4 if h == 1 else 2
                            if dc % nb == nb - 1:  # batched DMA
                                nc.sync.dma_start(
                                    out_r[:, dc - nb + 1:dc + 1, qs],
                                    obt[:, dc - nb + 1:dc + 1,
                                        h * HQ:(h + 1) * HQ])
                        yield task

            # ---------------- attention sweep ----------------
            def attention_qt(qt, fillers, parity_order=(0, 1)):
                """Causal attention for q-tile qt, both parities; pulls one
                filler thunk (proj/outproj work) every block."""
                nblk = 4 * qt + 4
                q0 = qt * QT

                npulls = 2 * (nblk + 2)
                state = {"done": 0, "credit": 0.0}
                rate = None

                def pull():
                    if not fillers:
                        return
                    if rate is None:
                        fillers.pop(0)()
                        return
                    state["credit"] += rate
                    while fillers and state["credit"] >= 1.0:
                        state["credit"] -= 1.0
                        fillers.pop(0)()

                rate = len(fillers) / float(npulls)
                for par in parity_order:  # 0: heads {0,2} rows 0:64
                    rows = slice(64 * par, 64 * par + 64)
                    ots = [ot_ps.tile([65, QT], F32, tag="ot", name="ot")
                           for _ in range(2)]
                    sts, pts, offs = {}, {}, {}

                    def scores(kb):
                        moff = max(0, (kb - 4 * qt) * KB)
                        n = QT - moff
                        st = st_ps.tile([128, 2, QT], F32, tag="st",
                                        name="st")
                        for s in range(2):
                            nc.tensor.matmul(
                                st[:, s, moff:QT],
                                kt2_sb[rows, kb * KB:(kb + 1) * KB],
                                qh_sb[rows, s, q0 + moff:q0 + QT],
                                start=True, stop=True)
                        pt = pt_pool.tile([128, 2, QT], BF16, tag="pt",
                                          name="pt")
                        nc.scalar.activation(
                            pt[:, :, moff:QT], st[:, :, moff:QT], EXP,
                            scale=float(SCALE), bias=zbias[:])
                        if kb >= 4 * qt:  # diagonal block: causal mask
                            nc.gpsimd.affine_select(
                                out=pt[:, :, moff:moff + KB],
                                in_=pt[:, :, moff:moff + KB],
                                compare_op=GE, fill=0.0, base=0,
                                pattern=[[0, 2], [1, KB]],
                                channel_multiplier=-1)
                        sts[kb], pts[kb], offs[kb] = st, pt, moff

                    def pv(kb):
                        pt, moff = pts.pop(kb), offs[kb]
                        del sts[kb]
                        for s in range(2):
                            nc.tensor.matmul(
                                ots[s][:, moff:QT],
                                vaug_sb[:, kb, 0:65],
                                pt[:, s, moff:QT],
                                start=(kb == 0), stop=(kb == nblk - 1))

                    scores(0)
                    if nblk > 1:
                        scores(1)
                    state["credit"] += 1.0
                    pull()
                    pull()
                    for kb in range(nblk):
                        if kb + 2 < nblk:
                            scores(kb + 2)
                        else:
                            pull()
                            pull()
                        pv(kb)
                        pull()
                    state["credit"] += 1.0
                    pull()

                    # normalize per slot: at = ot[0:64] * (1 / ot[64]);
                    # in the tail parity, split by (half, slot) h-major so
                    # the tail out-projection's first half starts early
                    tailpar = (qt == NQT - 1 and par == parity_order[1])
                    pieces = [(h, s) for h in range(2) for s in range(2)] \
                        if tailpar else [(None, s) for s in range(2)]
                    atmp = None
                    if par == 1:
                        atmp = ssb.tile([64, 2, QT], BF16, tag="atmp",
                                        name="atmp")
                    for h, s in pieces:
                        hs = slice(0, QT) if h is None else \
                            slice(h * (QT // 2), (h + 1) * (QT // 2))
                        nw = hs.stop - hs.start
                        lrec = ssb.tile([1, QT], F32, tag="lrec",
                                        name="lrec")[:, 0:nw]
                        nc.vector.reciprocal(lrec, ots[s][64:65, hs])
                        lrecb = ssb.tile([64, QT], F32, tag="lrecb",
                                         name="lrecb")[:, 0:nw]
                        nc.gpsimd.partition_broadcast(lrecb, lrec)
                        if par == 0:
                            nc.vector.tensor_tensor(
                                at_sb[0:64, s, q0 + hs.start:q0 + hs.stop],
                                ots[s][0:64, hs], lrecb, MULT)
                        else:
                            nc.vector.tensor_tensor(atmp[:, s, hs],
                                                    ots[s][0:64, hs],
                                                    lrecb, MULT)
                    if par == 1:
                        nc.sync.dma_start(at_sb[64:128, :, q0:q0 + QT],
                                          atmp[:])

            # ---------------- emit the pipeline ----------------
            for task in proj_chunk_tasks(0):
                task()
            for c in range(NQT):
                if c + 1 < NQT:
                    cs = slice((c + 1) * QT, (c + 2) * QT)
                    nc.sync.dma_start(xt_sb[:, 0:4, cs], xt_r[:, 0:4, cs])
                    nc.sync.dma_start(xt_sb[:, 4:ND, cs], xt_r[:, 4:ND, cs])
                fillers = []
                if c + 1 < NQT:
                    fillers.extend(proj_chunk_tasks(c + 1))
                if c == 1:
                    fillers.extend(outproj_tasks(0))
                elif c == 3:
                    op1 = list(outproj_tasks(1))
                    op2 = list(outproj_tasks(2))
                    # interleave the two qts' outproj thunks
                    for a, b in zip(op1, op2):
                        fillers.extend((a, b))
                attention_qt(c, fillers,
                             parity_order=(1, 0) if c == NQT - 1 else (0, 1))
                for f in fillers:  # anything not pulled during the sweep
                    f()
            for task in outproj_tasks(NQT - 1, tail=True):
                task()
    nc.compile()
    return nc


_NC_CACHE = None


def _get_nc():
    global _NC_CACHE
    if _NC_CACHE is None:
        _NC_CACHE = build_nc()
    return _NC_CACHE


# Q/K head dims are stored permuted so the rotate-half partner of the dim
# at lane p sits at lane (p+16)%32 within p's 32-lane group (DVE
# stream_shuffle reach). Scores are invariant to the shared permutation.
PERM64 = np.concatenate([np.arange(0, 16), np.arange(32, 48),
                         np.arange(16, 32), np.arange(48, 64)])


def _rope_tables():
    theta = 10000.0 ** (-(np.arange(HALF, dtype=np.float64) / HALF))
    pos = np.arange(S, dtype=np.float64)
    freqs = pos[:, None] * theta[None, :]  # [S, 32]
    cos1 = np.cos(freqs).T  # [32, S]
    sin1 = np.sin(freqs).T
    sign = np.where(PERM64 < HALF, -1.0, 1.0)[:, None]
    cos64 = cos1[PERM64 % HALF]
    sin64 = sign * sin1[PERM64 % HALF]
    cosq = np.tile(cos64, (2, 1)).astype(BF)  # [128, S]
    sinq = np.tile(sin64, (2, 1)).astype(BF)
    return np.ascontiguousarray(cosq), np.ascontiguousarray(sinq)


def _permute_qk_cols(w):
    """Permute each head's 64 columns of a [D, n*64] Q/K weight slice."""
    n = w.shape[1] // 64
    idx = np.concatenate([h * 64 + PERM64 for h in range(n)])
    return w[:, idx]


def _tp(w):
    """[t*128, m] -> [128, t, m] (pre-rearranged for big DMA descriptors)."""
    t = w.shape[0] // 128
    return np.ascontiguousarray(
        w.reshape(t, 128, w.shape[1]).transpose(1, 0, 2))


def make_in_maps(x, Wq, Wk, Wv, Wo):
    cosq, sinq = _rope_tables()
    xts = [np.ascontiguousarray(x[b].T.astype(BF)) for b in range(B)]
    wslices = {}
    in_maps = []
    for c in range(8):
        b, g = divmod(c, 4)
        if g not in wslices:
            wslices[g] = {
                "wq0": _tp(_permute_qk_cols(
                    Wq[:, g * GDIM:g * GDIM + 128]).astype(BF)),
                "wq1": _tp(_permute_qk_cols(
                    Wq[:, g * GDIM + 128:(g + 1) * GDIM]).astype(BF)),
                "wvk": _tp(np.concatenate(
                    [Wv[:, g * DK:(g + 1) * DK],
                     _permute_qk_cols(Wk[:, g * DK:(g + 1) * DK])],
                    axis=1).astype(BF)),
                "wo": _tp(Wo[g * GDIM:(g + 1) * GDIM, :].astype(BF)),
            }
        in_maps.append({
            "xt": xts[b], **wslices[g],
            "cosq": cosq, "sinq": sinq,
        })
    return in_maps


def kernel(x, mask, Wq, bq, Wk, bk, Wv, bv, Wo, bo):
    x = np.asarray(x, dtype=np.float32)
    mask = np.asarray(mask)
    Wq, Wk, Wv, Wo = (np.asarray(w, dtype=np.float32) for w in (Wq, Wk, Wv, Wo))
    bq, bk, bv, bo = (np.asarray(b, dtype=np.float32) for b in (bq, bk, bv, bo))

    assert np.array_equal(
        np.asarray(mask[0, 0]), np.tril(np.ones((S, S), mask.dtype))
    ), "kernel specialized for the causal mask"
    assert not bq.any() and not bk.any(), (
        "nonzero bq/bk not supported (cannot be folded outside RoPE)"
    )

    in_maps = make_in_maps(x, Wq, Wk, Wv, Wo)
    res = run_bass_kernel_spmd(_get_nc(), in_maps, list(range(8)))
    out = np.zeros((B, S, D), dtype=np.float32)
    for c in range(8):
        out[c // 4] += res.results[c]["outT"].astype(np.float32).T
    # host-side fold of the (structurally zero) v/out biases:
    # rows of softmax(P) sum to 1, so P @ (V + 1 bv^T) @ Wo + bo
    #   = P@V@Wo + sum_g bv_g_expanded @ Wo_g + bo
    corr = bo.astype(np.float64).copy()
    if bv.any():
        for g in range(KH):
            bv_full = np.tile(bv[g * DK:(g + 1) * DK], REP)
            corr = corr + bv_full.astype(np.float64) @ Wo[g * GDIM:(g + 1) * GDIM]
    if corr.any():
        out = out + corr[None, None, :].astype(np.float32)
    return out
